# revision 77
# baseline (speedup 1.0000x reference)
"""DeepSet (segment_reduce) Trainium2 Bass kernel, v3.

Computes, for each batch row b of x [B, 544]:
    s_i = x[:, :16]; s_g = x[:, 16:32]; s_js = x[:, 32:].reshape(B, 32, 16)
    h   = relu(s_js @ W0 + b0); h = relu(h @ W1 + b1); h = h @ W2 + b2
    summ = h.sum(axis=1)
    out = relu([s_i, s_g, summ] @ RW0 + rb0) @ RW1 + rb1        # [B, 16]

Sharding: pure data-parallel over 8 NeuronCores (batch 16384 -> 8 x 2048),
weights replicated.

v3 changes (vs v2's 104us -> 97.6us):
- The kernel is drain-bound: every neighbor's 128-dim activation must cross
  PSUM->SBUF through ACT or DVE twice (L0-relu, L1-relu) = 131k columns,
  a hard ~75us two-lane floor (Pool/GPSIMD cannot touch PSUM; DMA cannot
  either; DVE 2x modes need all-SBUF or all-16-bit operands).
- All three phi layers run as fp8 DoubleRow (0.5 PE cyc/col) with
  weight-corrected hi|res stationary packs at 8x scale (data-corrected DR
  measured 3x worse); drains write fp8 moving operands directly.
  PE drops 72us -> 45us and the PSUM slot cycle shortens.
- rho1 is computed TRANSPOSED (stationary = r0s batch-blocks, moving =
  rho_w1 slices, bias via a rank-1 ones x rb1row matmul) so the PE
  transposes, the ACT bias pass and the extra oN copy all disappear;
  outT reuses the drained acc PSUM region (no extra bank).
- s_i/s_g copy into X96 moved to the idle Pool (GPSIMD) engine.
- Fixed drain lanes: h0 drains on DVE (8-deep exec queue reorders around
  the fresh-matmul dependency), h1 drains on ACT (depth-0, strictly
  serial, so only the steady one-step-old stream lives there).  Any work
  inserted into ACT's stream measurably stalls the pipeline.
- rho stages are interleaved into the next super-block's pair loop,
  one fine-grained stage per step, starting one step late (spacer), so
  the boundary burst never parks the in-order PE queue.
- Accuracy 9.0e-3 vs the 2e-2 gate (bit-exact with the numpy fp8 model).
"""

import numpy as np
import ml_dtypes
from contextlib import ExitStack

F8NP = ml_dtypes.float8_e4m3

STATE_DIM = 16
N_NEIGH = 32
HIDDEN = 64
XCOLS = (2 + N_NEIGH) * STATE_DIM  # 544
B_FULL = 16384
N_CORES = 8
BC = B_FULL // N_CORES  # 2048 rows per core
SB = 512                # batch rows per super-block (matmul N)
NSB_FULL = BC // SB     # 4
W0SCALE = 8.0
W1SCALE = 8.0
W2SCALE = 8.0

_CACHE = {}

# Drain-lane assignment (selected by offline TimelineSim sweep).  h0 drains
# ride DVE (its 8-deep exec queue absorbs the fresh-A dependency), h1 drains
# ride ACT (depth-0, strictly serial, so only the steady D-stream lives
# there).  rho work is fine-grained and spaced so it never bursts into the
# steady pipeline.
_VARIANT = {"b_lane": "dve", "d_lane": "act", "steal": set(),
            "steal_lane": "act", "b_steal": set(), "summ_lane": "dve",
            "relu0_lanes": ("act", "act"), "pop_every": 1, "rho_spacer": 1,
            "split_tail": True, "b_split": set(), "rho_fine": True,
            "d_split": set(), "relu0_lanes_last": ("act", "dve"),
            "pool_wc": False, "pool_wd": False, "summ_split": False,
            "relu0_quarters": False}

_WOFF_A = {
    "w0": (0, 2048),      # 8 variants x [hi|res] planes of 8*W0
    "w2": (2048, 2176),   # [hi|res] planes of 8*W2  [128, 2, 64]
    "w1": (2176, 2432),   # [hi|res] planes of 8*W1  [128, 2, 128]
}
WA_COLS = 2432
_WOFF_C = {
    "b0s": (0, 1, 128),
    "b1s": (1, 2, 128),
    "b2s": (2, 3, HIDDEN),
    "rb0a": (3, 4, 128),
    "rb0b": (4, 5, 128),
}
WC_COLS = 5
_WOFF_D = {
    "rw0a": (0, 128, 96),
    "rw0b": (128, 256, 96),
    "rw1a": (256, 272, 128),
    "rw1b": (272, 288, 128),
    "ones64": (288, 352, 1),
    "rb1row": (352, 368, 1),
}
WD_COLS = 368


def build_nc(n_sb=NSB_FULL):
    import concourse.bass as bass
    import concourse.bacc as bacc
    import concourse.tile as tile
    import concourse.mybir as mybir

    f32 = mybir.dt.float32
    f32r = mybir.dt.float32r
    f8 = mybir.dt.float8e4
    AF = mybir.ActivationFunctionType
    ALU = mybir.AluOpType
    DR = mybir.MatmulPerfMode.DoubleRow

    rows = n_sb * SB
    n_groups = 16 * n_sb  # pair of neighbors per group
    nc = bacc.Bacc("TRN2", target_bir_lowering=False, debug=False)

    xs8 = nc.declare_dram_parameter("xs8", [512, rows], f8, isOutput=False)
    xg = nc.declare_dram_parameter("xg", [32, rows], f32, isOutput=False)
    wA = nc.declare_dram_parameter("wA", [128, WA_COLS], f8, isOutput=False)
    wC = nc.declare_dram_parameter("wC", [128, WC_COLS], f32, isOutput=False)
    wD = nc.declare_dram_parameter("wD", [128, WD_COLS], f32r, isOutput=False)
    y = nc.declare_dram_parameter("y", [rows, 16], f32, isOutput=True)
    yv = y.rearrange("(b p) f -> p b f", p=64)  # [64, 8*n_sb, 16]

    with tile.TileContext(nc) as tc, ExitStack() as ctx:
        wp = ctx.enter_context(tc.tile_pool(name="wts", bufs=1))
        # DMA-written tiles get dedicated slots (single-sync-wait rule).
        pxs = ctx.enter_context(tc.tile_pool(name="xs", bufs=4 * n_sb))
        ph0 = ctx.enter_context(tc.tile_pool(name="h0", bufs=6))
        pr1 = ctx.enter_context(tc.tile_pool(name="r1", bufs=6))
        pX96 = ctx.enter_context(tc.tile_pool(name="X96", bufs=n_sb))
        pr0 = ctx.enter_context(tc.tile_pool(name="r0", bufs=2))
        poN = ctx.enter_context(tc.tile_pool(name="oN", bufs=1))
        pA = ctx.enter_context(tc.tile_pool(name="pA", bufs=3, space="PSUM"))
        qAcc = ctx.enter_context(tc.tile_pool(name="qAcc", bufs=2, space="PSUM"))

        # startup DMAs, ordered so the L0 pipeline starts earliest:
        # A_0 needs wA + xs(0,0); B_0 needs wC; C_0 needs wD.
        twA0 = wp.tile([128, WA_COLS], f8, tag="wA0")
        nc.sync.dma_start(twA0[:], wA[:])
        xst = {}

        def load_xs(sb, js=range(4)):
            for j in js:
                t = pxs.tile([128, SB], f8, tag="xs", name=f"xs{sb}_{j}")
                nc.sync.dma_start(
                    t[:], xs8[128 * j:128 * (j + 1), SB * sb:SB * (sb + 1)])
                xst[(sb, j)] = t

        load_xs(0, js=(0,))
        twC = wp.tile([128, WC_COLS], f32, tag="wC")
        (nc.gpsimd if _VARIANT["pool_wc"] else nc.sync).dma_start(
            twC[:], wC[:])
        twD = wp.tile([128, WD_COLS], f32r, tag="wD")
        (nc.gpsimd if _VARIANT["pool_wd"] else nc.sync).dma_start(
            twD[:], wD[:])
        load_xs(0, js=(1, 2, 3))
        txg = wp.tile([32, rows], f32, tag="xg")
        nc.sync.dma_start(txg[:], xg[:])
        for sb in range(1, n_sb):
            load_xs(sb)

        def wc(name):
            c0, c1, p = _WOFF_C[name]
            return twC[0:p, c0:c1]

        def wd(name):
            c0, c1, p = _WOFF_D[name]
            return twD[0:p, c0:c1]

        def w0var(m):  # [128, 2, 128] fp8 hi|res planes of 8*W0 variant m
            return twA0[:, 256 * m:256 * (m + 1)].rearrange(
                "p (two c) -> p two c", two=2)

        w2pack = twA0[:, 2048:2176].rearrange(
            "p (two c) -> p two c", two=2)  # [128, 2, 64]
        w1pack = twA0[:, 2176:2432].rearrange(
            "p (two c) -> p two c", two=2)  # [128, 2, 128]

        trw0a, trw0b = wd("rw0a"), wd("rw0b")
        trw1a, trw1b = wd("rw1a"), wd("rw1b")
        tones, trb1row = wd("ones64"), wd("rb1row")
        tb0, tb1, tb2s = wc("b0s"), wc("b1s"), wc("b2s")
        trb0a, trb0b = wc("rb0a"), wc("rb0b")

        # Single-sync-wait discipline: each engine observes the startup DMAs
        # it depends on through dummy single-wait ops before real work.
        prev = {"pe": None, "act": None, "dve": None, "pool": None}

        def observe(k, ins):
            if prev[k] is not None:
                tile.add_dep_helper(ins.ins, prev[k].ins, sync=False,
                                    reason="startup order")
            prev[k] = ins

        dqA = qAcc.tile([1, 1], f32, tag="qAcc")
        observe("pe", nc.tensor.matmul(
            dqA[0:1, 0:1], twA0[0:1, 0:4].bitcast(f32),
            twA0[0:1, 0:4].bitcast(f32), start=True, stop=True))
        da0 = wp.tile([1, 1], f32, tag="dumA0")
        observe("act", nc.scalar.copy(da0[0:1, 0:1], twC[0:1, 0:1]))
        dv0 = wp.tile([1, 1], f32, tag="dumV0")
        observe("dve", nc.vector.tensor_copy(dv0[0:1, 0:1], twC[0:1, 0:1]))

        # Lane assignment knobs (swept offline; see _VARIANT).
        load = {"act": 0.0, "dve": 0.0}

        def lane_pick(n):
            c_act = n / 1.2 + 185.0
            c_dve = n * 1.0416667 + 125.0
            if load["act"] + c_act <= load["dve"] + c_dve:
                load["act"] += c_act
                return "act"
            load["dve"] += c_dve
            return "dve"

        def drain_relu(dst, src, bias, eng=None):
            if eng is None:
                eng = lane_pick(dst.shape[-1])
            if eng == "act":
                observe("act", nc.scalar.activation(
                    dst, src, AF.Relu, bias=bias))
            else:
                observe("dve", nc.vector.tensor_scalar(
                    dst, src, bias, 0.0, ALU.add, ALU.max))

        # pre-allocate X96 tiles; fill s_i/s_g halves on the idle Pool engine
        X96s = []
        for sb in range(n_sb):
            t = pX96.tile([96, SB], f32r, tag="X96", name=f"X96_{sb}")
            X96s.append(t)
            observe("pool", nc.gpsimd.tensor_copy(
                t[64:96, :], txg[:, SB * sb:SB * (sb + 1)]))

        oN = poN.tile([64, 128 * n_sb], f32, tag="oN")

        state = {}
        accs = {}
        r0ss = {}
        deferred = []

        def emit_A(g):  # L0: fp8 DoubleRow, 2 neighbors -> hp [128,1024]
            sb, i = divmod(g, 16)
            hp = pA.tile([128, 1024], f32, tag="pA", name=f"hp{g}")
            for k in (0, 1):
                n = 2 * i + k
                observe("pe", nc.tensor.matmul(
                    hp[:, 512 * k:512 * (k + 1)], w0var(n % 8),
                    xst[(sb, n // 8)][:].unsqueeze(1).broadcast_to(
                        [128, 2, SB]),
                    start=True, stop=True, perf_mode=DR))
            state[g] = {"hp": hp}

        def emit_B(g, eng="__default__"):  # h0 drain: relu+bias -> fp8 SBUF
            if eng == "__default__":
                eng = _VARIANT["b_lane"]
            h0s = ph0.tile([128, 1024], f8, tag="h0", name=f"h0s{g}")
            hp = state[g]["hp"]
            if g in _VARIANT["b_split"]:
                # split across both lanes for load balance: same deps as a
                # whole drain, so no queue-position hazard on either lane
                drain_relu(h0s[:, 0:512], hp[:, 0:512], tb0[:, 0:1],
                           "act" if eng == "dve" else "dve")
                drain_relu(h0s[:, 512:1024], hp[:, 512:1024], tb0[:, 0:1],
                           eng)
            else:
                drain_relu(h0s[:], hp[:], tb0[:, 0:1], eng)
            state[g]["h0s"] = h0s

        def emit_C(g):  # L1: fp8 DoubleRow; h1 reuses the L0 PSUM tile (WAR)
            if g == 0:
                dqD = qAcc.tile([1, 1], f32, tag="qAcc")
                observe("pe", nc.tensor.matmul(
                    dqD[0:1, 0:1], twD[0:1, 0:1].bitcast(f32),
                    twD[0:1, 0:1].bitcast(f32), start=True, stop=True))
            h0s = state[g]["h0s"]
            h1p = state[g]["hp"]
            for k in (0, 1):
                observe("pe", nc.tensor.matmul(
                    h1p[:, 512 * k:512 * (k + 1)], w1pack,
                    h0s[:, 512 * k:512 * (k + 1)].unsqueeze(1).broadcast_to(
                        [128, 2, 512]),
                    start=True, stop=True, perf_mode=DR))

        STEAL = _VARIANT["steal"]

        def emit_D(g, eng="__default__"):  # h1 drain: relu+bias -> fp8 SBUF
            if eng == "__default__":
                eng = _VARIANT["d_lane"]
            r1 = pr1.tile([128, 1024], f8, tag="r1", name=f"r1s{g}")
            hp = state[g]["hp"]
            if g in _VARIANT["d_split"]:
                drain_relu(r1[:, 0:512], hp[:, 0:512], tb1[:, 0:1],
                           "act" if eng == "dve" else "dve")
                drain_relu(r1[:, 512:1024], hp[:, 512:1024], tb1[:, 0:1],
                           eng)
            else:
                drain_relu(r1[:], hp[:], tb1[:, 0:1], eng)
            state[g]["r1"] = r1

        def emit_E(g):  # L2: fp8 DoubleRow, accumulate neighbor sum in PSUM
            sb, i = divmod(g, 16)
            if i == 0:
                accs[sb] = qAcc.tile([HIDDEN, SB], f32, tag="qAcc",
                                     name=f"acc{sb}")
            acc = accs[sb]
            r1 = state[g]["r1"]
            for k in (0, 1):
                n = 2 * i + k
                observe("pe", nc.tensor.matmul(
                    acc[:], w2pack,
                    r1[:, 512 * k:512 * (k + 1)].unsqueeze(1).broadcast_to(
                        [128, 2, 512]),
                    start=(n == 0), stop=(n == N_NEIGH - 1), perf_mode=DR))
            del state[g]

        def s_summ1(sb, c0, cn, eng):
            dst = X96s[sb][0:64, c0:c0 + cn]
            src = accs[sb][:, c0:c0 + cn]
            sc = 1.0 / (W0SCALE * W1SCALE * W2SCALE)
            if eng == "act":
                observe("act", nc.scalar.activation(
                    dst, src, AF.Identity, bias=tb2s[:, 0:1], scale=sc))
            else:
                observe("dve", nc.vector.tensor_scalar(
                    dst, src, sc, tb2s[:, 0:1], ALU.mult, ALU.add))

        def s_summ(sb, c0=0, cn=SB):  # X96[0:64] = acc/512 + 32*b2
            if _VARIANT["summ_split"]:
                h = cn // 2
                s_summ1(sb, c0, h, "dve")
                s_summ1(sb, c0 + h, h, "act")
            else:
                s_summ1(sb, c0, cn, _VARIANT["summ_lane"])

        def s_rho0(sb, c0=0, cn=SB):
            if ("r0p", sb) not in state:
                state[("r0p", sb)] = pA.tile([128, 1024], f32, tag="pA",
                                             name=f"r0p{sb}")
            r0p = state[("r0p", sb)]
            observe("pe", nc.tensor.matmul(
                r0p[:, c0:c0 + cn], trw0a, X96s[sb][:, c0:c0 + cn],
                start=True, stop=True))
            observe("pe", nc.tensor.matmul(
                r0p[:, 512 + c0:512 + c0 + cn], trw0b,
                X96s[sb][:, c0:c0 + cn], start=True, stop=True))

        def s_relu0(sb, c0=0, cn=SB):
            r0p = state[("r0p", sb)]
            if sb not in r0ss:
                r0ss[sb] = pr0.tile([128, 1024], f32r, tag="r0",
                                    name=f"r0s{sb}")
            r0s = r0ss[sb]
            lanes = (_VARIANT["relu0_lanes_last"] if sb == n_sb - 1
                     else _VARIANT["relu0_lanes"])
            for half, bias, eng in ((0, trb0a, lanes[0]),
                                    (1, trb0b, lanes[1])):
                drain_relu(r0s[:, 512 * half + c0:512 * half + c0 + cn],
                           r0p[:, 512 * half + c0:512 * half + c0 + cn],
                           bias[:, 0:1], eng)

        def s_relu0h(sb, half):  # one 512-col feature half of relu0
            r0p = state[("r0p", sb)]
            if sb not in r0ss:
                r0ss[sb] = pr0.tile([128, 1024], f32r, tag="r0",
                                    name=f"r0s{sb}")
            bias = trb0a if half == 0 else trb0b
            drain_relu(r0ss[sb][:, 512 * half:512 * (half + 1)],
                       r0p[:, 512 * half:512 * (half + 1)],
                       bias[:, 0:1], _VARIANT["relu0_lanes"][half])

        def s_relu0q(sb, half, ch):  # one [128,256] quarter of relu0
            r0p = state[("r0p", sb)]
            if sb not in r0ss:
                r0ss[sb] = pr0.tile([128, 1024], f32r, tag="r0",
                                    name=f"r0s{sb}")
            bias = trb0a if half == 0 else trb0b
            o = 512 * half + 256 * ch
            drain_relu(r0ss[sb][:, o:o + 256], r0p[:, o:o + 256],
                       bias[:, 0:1], _VARIANT["relu0_lanes"][half])

        def s_rho1T(sb, c0=0, cn=SB):
            # transposed rho1: out[batch, feat] = r0s_blk^T @ rw1 + 1 x rb1.
            # outT reuses the drained acc(sb) PSUM region (dead after
            # s_summ), so it costs no PSUM slot and the acc pool alternates
            # cleanly between consecutive super-blocks.
            r0s = r0ss[sb]
            outT = accs[sb][0:64, 0:128]
            for blk in range(c0 // 64, (c0 + cn) // 64):
                reg = outT[:, 16 * blk:16 * (blk + 1)]
                b0 = 64 * blk
                observe("pe", nc.tensor.matmul(
                    reg, tones, trb1row, start=True, stop=False))
                observe("pe", nc.tensor.matmul(
                    reg, r0s[:, b0:b0 + 64], trw1a,
                    start=False, stop=False))
                observe("pe", nc.tensor.matmul(
                    reg, r0s[:, 512 + b0:512 + b0 + 64], trw1b,
                    start=False, stop=True))

        def s_out(sb, c0=0, cn=SB):
            outT = accs[sb][0:64, 0:128]
            b0, b1 = c0 // 64, (c0 + cn) // 64
            observe("act", nc.scalar.copy(
                oN[:, 128 * sb + 16 * b0:128 * sb + 16 * b1],
                outT[:, 16 * b0:16 * b1]))
            nc.sync.dma_start(
                yv[:, 8 * sb + b0:8 * sb + b1, :],
                oN[:, 128 * sb + 16 * b0:128 * sb + 16 * b1].rearrange(
                    "p (b f) -> p b f", f=16))

        # Newest work first each step (A/B, then C/D, then E) keeps the
        # in-order PE queue from blocking on lane results.  At super-block
        # boundaries E + s_summ are hoisted to the front instead so s_summ
        # lands in its lane queue ahead of the step's main drains and the
        # acc PSUM slot frees before the next super-block's first L2.
        def emit_rho_queue(sb):
            deferred.extend([None] * _VARIANT["rho_spacer"])
            if sb < n_sb - 1 and _VARIANT["rho_fine"]:
                # interior super-blocks: fine-grained pops so the rho PE/ACT
                # work never bursts into the steady A/C stream
                h = SB // 2
                if _VARIANT["relu0_quarters"]:
                    deferred.extend([
                        lambda: s_rho0(sb),
                        lambda: s_relu0q(sb, 0, 0),
                        lambda: s_relu0q(sb, 0, 1),
                        lambda: s_relu0q(sb, 1, 0),
                        lambda: s_relu0q(sb, 1, 1),
                        lambda: s_rho1T(sb, 0, h),
                        lambda: s_rho1T(sb, h, h),
                        lambda: s_out(sb),
                    ])
                else:
                    deferred.extend([
                        lambda: s_rho0(sb),
                        lambda: s_relu0h(sb, 0),
                        lambda: s_relu0h(sb, 1),
                        lambda: s_rho1T(sb, 0, h),
                        lambda: s_rho1T(sb, h, h),
                        lambda: s_out(sb),
                    ])
            else:
                deferred.extend([
                    lambda: s_rho0(sb),
                    lambda: s_relu0(sb),
                    lambda: s_rho1T(sb),
                    lambda: s_out(sb),
                ])

        B_STEAL = _VARIANT["b_steal"]
        stolen = []
        for step in range(n_groups + 2):
            if step < n_groups:
                emit_A(step)
                if step not in B_STEAL:
                    emit_B(step)
            if stolen:  # one-step-delayed h1 drain, rebalanced cross-lane
                emit_D(stolen.pop(), eng=_VARIANT["steal_lane"])
            if 1 <= step <= n_groups:
                emit_C(step - 1)
                if step - 1 in STEAL:
                    stolen.append(step - 1)
                else:
                    emit_D(step - 1)
            if step < n_groups and step in B_STEAL:
                # stolen h0 drain: cross-lane, after the D of this step so
                # the receiving lane never parks on the fresh A matmuls
                emit_B(step, eng=_VARIANT["steal_lane"])
            if step >= 2:
                g = step - 2
                emit_E(g)
                if g % 16 == 15:
                    s_summ(g // 16)
                    emit_rho_queue(g // 16)
            if deferred and step % _VARIANT["pop_every"] == 0:
                fn = deferred.pop(0)
                if fn is not None:
                    fn()
        while deferred:
            fn = deferred.pop(0)
            if fn is not None:
                fn()

    nc.compile()
    return nc


def prep_inputs(inputs):
    """Host-side layout prep: transposed/quantized x + packed weights."""
    f = np.float32

    def q8(a):
        return np.asarray(a, f).astype(F8NP)

    w0 = np.asarray(inputs["phi_w0"], f) * W0SCALE   # [16, 128]
    wAf = np.zeros((128, WA_COLS), f)
    for m in range(8):
        var = np.zeros((128, 128), f)
        var[16 * m:16 * m + 16, :] = w0
        hi = q8(var)
        res = q8(var - hi.astype(f))
        wAf[:, 256 * m:256 * m + 128] = hi.astype(f)
        wAf[:, 256 * m + 128:256 * m + 256] = res.astype(f)
    w2 = np.asarray(inputs["phi_w2"], f) * W2SCALE   # [128, 64]
    hi2 = q8(w2)
    res2 = q8(w2 - hi2.astype(f))
    wAf[:, 2048:2112] = hi2.astype(f)
    wAf[:, 2112:2176] = res2.astype(f)
    w1 = np.asarray(inputs["phi_w1"], f) * W1SCALE   # [128, 128]
    hi1 = q8(w1)
    res1 = q8(w1 - hi1.astype(f))
    wAf[:, 2176:2304] = hi1.astype(f)
    wAf[:, 2304:2432] = res1.astype(f)
    wA = wAf.astype(F8NP)

    rho_w0 = np.asarray(inputs["rho_w0"], f)
    rho_w0 = np.concatenate([rho_w0[32:96], rho_w0[0:32]], axis=0)
    rho_w1 = np.asarray(inputs["rho_w1"], f)
    partsD = {
        "rw0a": rho_w0[:, :128],
        "rw0b": rho_w0[:, 128:],
        "rw1a": rho_w1[:128],
        "rw1b": rho_w1[128:],
        "ones64": np.ones((1, 64), f),
        "rb1row": np.asarray(inputs["rho_b1"], f).reshape(1, 16),
    }
    partsC = {
        "b0s": (W0SCALE * np.asarray(inputs["phi_b0"], f)).reshape(128, 1),
        "b1s": (W0SCALE * W1SCALE
                * np.asarray(inputs["phi_b1"], f)).reshape(128, 1),
        "b2s": (N_NEIGH * np.asarray(inputs["phi_b2"], f)).reshape(HIDDEN, 1),
        "rb0a": np.asarray(inputs["rho_b0"], f)[:128].reshape(128, 1),
        "rb0b": np.asarray(inputs["rho_b0"], f)[128:].reshape(128, 1),
    }
    wCm = np.zeros((128, WC_COLS), f)
    for name, (c0, c1, p) in _WOFF_C.items():
        arr = partsC[name]
        assert arr.shape == (p, c1 - c0), (name, arr.shape)
        wCm[:p, c0:c1] = arr
    wDm = np.zeros((128, WD_COLS), f)
    for name, (c0, c1, p) in _WOFF_D.items():
        arr = partsD[name]
        assert arr.shape == (p, c1 - c0), (name, arr.shape)
        wDm[:p, c0:c1] = arr
    wts = {"wA": wA, "wC": wCm, "wD": wDm}

    x = np.asarray(inputs["x"], f)
    assert x.shape == (B_FULL, XCOLS)
    in_maps = []
    for c in range(N_CORES):
        xT = np.ascontiguousarray(x[c * BC:(c + 1) * BC].T)  # [544, BC]
        in_maps.append({
            "xs8": np.ascontiguousarray(xT[32:544]).astype(F8NP),
            "xg": np.ascontiguousarray(xT[0:32]),
            **wts,
        })
    return in_maps


def prep_weights(inputs):  # kept for test.py compatibility
    return prep_inputs(inputs)[0]


def kernel(**inputs):
    from concourse.bass_utils import run_bass_kernel_spmd

    if "nc" not in _CACHE:
        _CACHE["nc"] = build_nc(NSB_FULL)
    nc = _CACHE["nc"]

    in_maps = prep_inputs(inputs)
    res = run_bass_kernel_spmd(nc, in_maps, list(range(N_CORES)))
    out = np.concatenate([res.results[c]["y"] for c in range(N_CORES)], axis=0)
    return out.astype(np.float32)


# revision 81
# speedup vs baseline: 1.0018x; 1.0018x over previous
"""DeepSet (segment_reduce) Trainium2 Bass kernel, v3.

Computes, for each batch row b of x [B, 544]:
    s_i = x[:, :16]; s_g = x[:, 16:32]; s_js = x[:, 32:].reshape(B, 32, 16)
    h   = relu(s_js @ W0 + b0); h = relu(h @ W1 + b1); h = h @ W2 + b2
    summ = h.sum(axis=1)
    out = relu([s_i, s_g, summ] @ RW0 + rb0) @ RW1 + rb1        # [B, 16]

Sharding: pure data-parallel over 8 NeuronCores (batch 16384 -> 8 x 2048),
weights replicated.

v3 changes (vs v2's 104us -> 97.6us):
- The kernel is drain-bound: every neighbor's 128-dim activation must cross
  PSUM->SBUF through ACT or DVE twice (L0-relu, L1-relu) = 131k columns,
  a hard ~75us two-lane floor (Pool/GPSIMD cannot touch PSUM; DMA cannot
  either; DVE 2x modes need all-SBUF or all-16-bit operands).
- All three phi layers run as fp8 DoubleRow (0.5 PE cyc/col) with
  weight-corrected hi|res stationary packs at 8x scale (data-corrected DR
  measured 3x worse); drains write fp8 moving operands directly.
  PE drops 72us -> 45us and the PSUM slot cycle shortens.
- rho1 is computed TRANSPOSED (stationary = r0s batch-blocks, moving =
  rho_w1 slices, bias via a rank-1 ones x rb1row matmul) so the PE
  transposes, the ACT bias pass and the extra oN copy all disappear;
  outT reuses the drained acc PSUM region (no extra bank).
- s_i/s_g copy into X96 moved to the idle Pool (GPSIMD) engine.
- Fixed drain lanes: h0 drains on DVE (8-deep exec queue reorders around
  the fresh-matmul dependency), h1 drains on ACT (depth-0, strictly
  serial, so only the steady one-step-old stream lives there).  Any work
  inserted into ACT's stream measurably stalls the pipeline.
- rho stages are interleaved into the next super-block's pair loop,
  one fine-grained stage per step, starting one step late (spacer), so
  the boundary burst never parks the in-order PE queue.
- Accuracy 9.0e-3 vs the 2e-2 gate (bit-exact with the numpy fp8 model).
"""

import numpy as np
import ml_dtypes
from contextlib import ExitStack

F8NP = ml_dtypes.float8_e4m3

STATE_DIM = 16
N_NEIGH = 32
HIDDEN = 64
XCOLS = (2 + N_NEIGH) * STATE_DIM  # 544
B_FULL = 16384
N_CORES = 8
BC = B_FULL // N_CORES  # 2048 rows per core
SB = 512                # batch rows per super-block (matmul N)
NSB_FULL = BC // SB     # 4
W0SCALE = 8.0
W1SCALE = 8.0
W2SCALE = 8.0

_CACHE = {}

# Drain-lane assignment (selected by offline TimelineSim sweep).  h0 drains
# ride DVE (its 8-deep exec queue absorbs the fresh-A dependency), h1 drains
# ride ACT (depth-0, strictly serial, so only the steady D-stream lives
# there).  rho work is fine-grained and spaced so it never bursts into the
# steady pipeline.
_VARIANT = {"b_lane": "dve", "d_lane": "act", "steal": set(),
            "steal_lane": "act", "b_steal": set(), "summ_lane": "dve",
            "relu0_lanes": ("act", "act"), "pop_every": 1, "rho_spacer": 1,
            "split_tail": True, "b_split": set(), "rho_fine": True,
            "d_split": set(), "relu0_lanes_last": ("act", "dve"),
            "pool_wc": False, "pool_wd": False, "summ_split": False,
            "relu0_quarters": False, "pe_warm": 4}

_WOFF_A = {
    "w0": (0, 2048),      # 8 variants x [hi|res] planes of 8*W0
    "w2": (2048, 2176),   # [hi|res] planes of 8*W2  [128, 2, 64]
    "w1": (2176, 2432),   # [hi|res] planes of 8*W1  [128, 2, 128]
}
WA_COLS = 2432
_WOFF_C = {
    "b0s": (0, 1, 128),
    "b1s": (1, 2, 128),
    "b2s": (2, 3, HIDDEN),
    "rb0a": (3, 4, 128),
    "rb0b": (4, 5, 128),
}
WC_COLS = 5
_WOFF_D = {
    "rw0a": (0, 128, 96),
    "rw0b": (128, 256, 96),
    "rw1a": (256, 272, 128),
    "rw1b": (272, 288, 128),
    "ones64": (288, 352, 1),
    "rb1row": (352, 368, 1),
}
WD_COLS = 368


def build_nc(n_sb=NSB_FULL):
    import concourse.bass as bass
    import concourse.bacc as bacc
    import concourse.tile as tile
    import concourse.mybir as mybir

    f32 = mybir.dt.float32
    f32r = mybir.dt.float32r
    f8 = mybir.dt.float8e4
    AF = mybir.ActivationFunctionType
    ALU = mybir.AluOpType
    DR = mybir.MatmulPerfMode.DoubleRow

    rows = n_sb * SB
    n_groups = 16 * n_sb  # pair of neighbors per group
    nc = bacc.Bacc("TRN2", target_bir_lowering=False, debug=False)

    xs8 = nc.declare_dram_parameter("xs8", [512, rows], f8, isOutput=False)
    xg = nc.declare_dram_parameter("xg", [32, rows], f32, isOutput=False)
    wA = nc.declare_dram_parameter("wA", [128, WA_COLS], f8, isOutput=False)
    wC = nc.declare_dram_parameter("wC", [128, WC_COLS], f32, isOutput=False)
    wD = nc.declare_dram_parameter("wD", [128, WD_COLS], f32r, isOutput=False)
    y = nc.declare_dram_parameter("y", [rows, 16], f32, isOutput=True)
    yv = y.rearrange("(b p) f -> p b f", p=64)  # [64, 8*n_sb, 16]

    with tile.TileContext(nc) as tc, ExitStack() as ctx:
        wp = ctx.enter_context(tc.tile_pool(name="wts", bufs=1))
        # DMA-written tiles get dedicated slots (single-sync-wait rule).
        pxs = ctx.enter_context(tc.tile_pool(name="xs", bufs=4 * n_sb))
        ph0 = ctx.enter_context(tc.tile_pool(name="h0", bufs=6))
        pr1 = ctx.enter_context(tc.tile_pool(name="r1", bufs=6))
        pX96 = ctx.enter_context(tc.tile_pool(name="X96", bufs=n_sb))
        pr0 = ctx.enter_context(tc.tile_pool(name="r0", bufs=2))
        poN = ctx.enter_context(tc.tile_pool(name="oN", bufs=1))
        pA = ctx.enter_context(tc.tile_pool(name="pA", bufs=3, space="PSUM"))
        qAcc = ctx.enter_context(tc.tile_pool(name="qAcc", bufs=2, space="PSUM"))

        # startup DMAs, ordered so the L0 pipeline starts earliest:
        # A_0 needs wA + xs(0,0); B_0 needs wC; C_0 needs wD.
        twA0 = wp.tile([128, WA_COLS], f8, tag="wA0")
        nc.sync.dma_start(twA0[:], wA[:])
        xst = {}

        def load_xs(sb, js=range(4)):
            for j in js:
                t = pxs.tile([128, SB], f8, tag="xs", name=f"xs{sb}_{j}")
                nc.sync.dma_start(
                    t[:], xs8[128 * j:128 * (j + 1), SB * sb:SB * (sb + 1)])
                xst[(sb, j)] = t

        load_xs(0, js=(0,))
        twC = wp.tile([128, WC_COLS], f32, tag="wC")
        (nc.gpsimd if _VARIANT["pool_wc"] else nc.sync).dma_start(
            twC[:], wC[:])
        twD = wp.tile([128, WD_COLS], f32r, tag="wD")
        (nc.gpsimd if _VARIANT["pool_wd"] else nc.sync).dma_start(
            twD[:], wD[:])
        load_xs(0, js=(1, 2, 3))
        txg = wp.tile([32, rows], f32, tag="xg")
        nc.sync.dma_start(txg[:], xg[:])
        for sb in range(1, n_sb):
            load_xs(sb)

        def wc(name):
            c0, c1, p = _WOFF_C[name]
            return twC[0:p, c0:c1]

        def wd(name):
            c0, c1, p = _WOFF_D[name]
            return twD[0:p, c0:c1]

        def w0var(m):  # [128, 2, 128] fp8 hi|res planes of 8*W0 variant m
            return twA0[:, 256 * m:256 * (m + 1)].rearrange(
                "p (two c) -> p two c", two=2)

        w2pack = twA0[:, 2048:2176].rearrange(
            "p (two c) -> p two c", two=2)  # [128, 2, 64]
        w1pack = twA0[:, 2176:2432].rearrange(
            "p (two c) -> p two c", two=2)  # [128, 2, 128]

        trw0a, trw0b = wd("rw0a"), wd("rw0b")
        trw1a, trw1b = wd("rw1a"), wd("rw1b")
        tones, trb1row = wd("ones64"), wd("rb1row")
        tb0, tb1, tb2s = wc("b0s"), wc("b1s"), wc("b2s")
        trb0a, trb0b = wc("rb0a"), wc("rb0b")

        # Single-sync-wait discipline: each engine observes the startup DMAs
        # it depends on through dummy single-wait ops before real work.
        prev = {"pe": None, "act": None, "dve": None, "pool": None}

        def observe(k, ins):
            if prev[k] is not None:
                tile.add_dep_helper(ins.ins, prev[k].ins, sync=False,
                                    reason="startup order")
            prev[k] = ins

        if _VARIANT["pe_warm"]:
            # p-state pre-warm: ~3us of throwaway matmuls on a zeroed tile
            # (no DMA dependency) so the first real L0 matmuls run at the
            # full 2.4GHz clock instead of the 1.2GHz mid p-state.
            wsrc = wp.tile([128, 512], f32, tag="warm")
            observe("pool", nc.gpsimd.memset(wsrc[:], 0.0))
            wout = qAcc.tile([1, 512], f32, tag="qAcc", name="warm")
            for _ in range(_VARIANT["pe_warm"]):
                observe("pe", nc.tensor.matmul(
                    wout[0:1, :], wsrc[:, 0:1].bitcast(f32r),
                    wsrc[:].bitcast(f32r), start=True, stop=True))
        dqA = qAcc.tile([1, 1], f32, tag="qAcc")
        observe("pe", nc.tensor.matmul(
            dqA[0:1, 0:1], twA0[0:1, 0:4].bitcast(f32),
            twA0[0:1, 0:4].bitcast(f32), start=True, stop=True))
        da0 = wp.tile([1, 1], f32, tag="dumA0")
        observe("act", nc.scalar.copy(da0[0:1, 0:1], twC[0:1, 0:1]))
        dv0 = wp.tile([1, 1], f32, tag="dumV0")
        observe("dve", nc.vector.tensor_copy(dv0[0:1, 0:1], twC[0:1, 0:1]))

        # Lane assignment knobs (swept offline; see _VARIANT).
        load = {"act": 0.0, "dve": 0.0}

        def lane_pick(n):
            c_act = n / 1.2 + 185.0
            c_dve = n * 1.0416667 + 125.0
            if load["act"] + c_act <= load["dve"] + c_dve:
                load["act"] += c_act
                return "act"
            load["dve"] += c_dve
            return "dve"

        def drain_relu(dst, src, bias, eng=None):
            if eng is None:
                eng = lane_pick(dst.shape[-1])
            if eng == "act":
                observe("act", nc.scalar.activation(
                    dst, src, AF.Relu, bias=bias))
            else:
                observe("dve", nc.vector.tensor_scalar(
                    dst, src, bias, 0.0, ALU.add, ALU.max))

        # pre-allocate X96 tiles; fill s_i/s_g halves on the idle Pool engine
        X96s = []
        for sb in range(n_sb):
            t = pX96.tile([96, SB], f32r, tag="X96", name=f"X96_{sb}")
            X96s.append(t)
            observe("pool", nc.gpsimd.tensor_copy(
                t[64:96, :], txg[:, SB * sb:SB * (sb + 1)]))

        oN = poN.tile([64, 128 * n_sb], f32, tag="oN")

        state = {}
        accs = {}
        r0ss = {}
        deferred = []

        def emit_A(g):  # L0: fp8 DoubleRow, 2 neighbors -> hp [128,1024]
            sb, i = divmod(g, 16)
            hp = pA.tile([128, 1024], f32, tag="pA", name=f"hp{g}")
            for k in (0, 1):
                n = 2 * i + k
                observe("pe", nc.tensor.matmul(
                    hp[:, 512 * k:512 * (k + 1)], w0var(n % 8),
                    xst[(sb, n // 8)][:].unsqueeze(1).broadcast_to(
                        [128, 2, SB]),
                    start=True, stop=True, perf_mode=DR))
            state[g] = {"hp": hp}

        def emit_B(g, eng="__default__"):  # h0 drain: relu+bias -> fp8 SBUF
            if eng == "__default__":
                eng = _VARIANT["b_lane"]
            h0s = ph0.tile([128, 1024], f8, tag="h0", name=f"h0s{g}")
            hp = state[g]["hp"]
            if g in _VARIANT["b_split"]:
                # split across both lanes for load balance: same deps as a
                # whole drain, so no queue-position hazard on either lane
                drain_relu(h0s[:, 0:512], hp[:, 0:512], tb0[:, 0:1],
                           "act" if eng == "dve" else "dve")
                drain_relu(h0s[:, 512:1024], hp[:, 512:1024], tb0[:, 0:1],
                           eng)
            else:
                drain_relu(h0s[:], hp[:], tb0[:, 0:1], eng)
            state[g]["h0s"] = h0s

        def emit_C(g):  # L1: fp8 DoubleRow; h1 reuses the L0 PSUM tile (WAR)
            if g == 0:
                dqD = qAcc.tile([1, 1], f32, tag="qAcc")
                observe("pe", nc.tensor.matmul(
                    dqD[0:1, 0:1], twD[0:1, 0:1].bitcast(f32),
                    twD[0:1, 0:1].bitcast(f32), start=True, stop=True))
            h0s = state[g]["h0s"]
            h1p = state[g]["hp"]
            for k in (0, 1):
                observe("pe", nc.tensor.matmul(
                    h1p[:, 512 * k:512 * (k + 1)], w1pack,
                    h0s[:, 512 * k:512 * (k + 1)].unsqueeze(1).broadcast_to(
                        [128, 2, 512]),
                    start=True, stop=True, perf_mode=DR))

        STEAL = _VARIANT["steal"]

        def emit_D(g, eng="__default__"):  # h1 drain: relu+bias -> fp8 SBUF
            if eng == "__default__":
                eng = _VARIANT["d_lane"]
            r1 = pr1.tile([128, 1024], f8, tag="r1", name=f"r1s{g}")
            hp = state[g]["hp"]
            if g in _VARIANT["d_split"]:
                drain_relu(r1[:, 0:512], hp[:, 0:512], tb1[:, 0:1],
                           "act" if eng == "dve" else "dve")
                drain_relu(r1[:, 512:1024], hp[:, 512:1024], tb1[:, 0:1],
                           eng)
            else:
                drain_relu(r1[:], hp[:], tb1[:, 0:1], eng)
            state[g]["r1"] = r1

        def emit_E(g):  # L2: fp8 DoubleRow, accumulate neighbor sum in PSUM
            sb, i = divmod(g, 16)
            if i == 0:
                accs[sb] = qAcc.tile([HIDDEN, SB], f32, tag="qAcc",
                                     name=f"acc{sb}")
            acc = accs[sb]
            r1 = state[g]["r1"]
            for k in (0, 1):
                n = 2 * i + k
                observe("pe", nc.tensor.matmul(
                    acc[:], w2pack,
                    r1[:, 512 * k:512 * (k + 1)].unsqueeze(1).broadcast_to(
                        [128, 2, 512]),
                    start=(n == 0), stop=(n == N_NEIGH - 1), perf_mode=DR))
            del state[g]

        def s_summ1(sb, c0, cn, eng):
            dst = X96s[sb][0:64, c0:c0 + cn]
            src = accs[sb][:, c0:c0 + cn]
            sc = 1.0 / (W0SCALE * W1SCALE * W2SCALE)
            if eng == "act":
                observe("act", nc.scalar.activation(
                    dst, src, AF.Identity, bias=tb2s[:, 0:1], scale=sc))
            else:
                observe("dve", nc.vector.tensor_scalar(
                    dst, src, sc, tb2s[:, 0:1], ALU.mult, ALU.add))

        def s_summ(sb, c0=0, cn=SB):  # X96[0:64] = acc/512 + 32*b2
            if _VARIANT["summ_split"]:
                h = cn // 2
                s_summ1(sb, c0, h, "dve")
                s_summ1(sb, c0 + h, h, "act")
            else:
                s_summ1(sb, c0, cn, _VARIANT["summ_lane"])

        def s_rho0(sb, c0=0, cn=SB):
            if ("r0p", sb) not in state:
                state[("r0p", sb)] = pA.tile([128, 1024], f32, tag="pA",
                                             name=f"r0p{sb}")
            r0p = state[("r0p", sb)]
            observe("pe", nc.tensor.matmul(
                r0p[:, c0:c0 + cn], trw0a, X96s[sb][:, c0:c0 + cn],
                start=True, stop=True))
            observe("pe", nc.tensor.matmul(
                r0p[:, 512 + c0:512 + c0 + cn], trw0b,
                X96s[sb][:, c0:c0 + cn], start=True, stop=True))

        def s_relu0(sb, c0=0, cn=SB):
            r0p = state[("r0p", sb)]
            if sb not in r0ss:
                r0ss[sb] = pr0.tile([128, 1024], f32r, tag="r0",
                                    name=f"r0s{sb}")
            r0s = r0ss[sb]
            lanes = (_VARIANT["relu0_lanes_last"] if sb == n_sb - 1
                     else _VARIANT["relu0_lanes"])
            for half, bias, eng in ((0, trb0a, lanes[0]),
                                    (1, trb0b, lanes[1])):
                drain_relu(r0s[:, 512 * half + c0:512 * half + c0 + cn],
                           r0p[:, 512 * half + c0:512 * half + c0 + cn],
                           bias[:, 0:1], eng)

        def s_relu0h(sb, half):  # one 512-col feature half of relu0
            r0p = state[("r0p", sb)]
            if sb not in r0ss:
                r0ss[sb] = pr0.tile([128, 1024], f32r, tag="r0",
                                    name=f"r0s{sb}")
            bias = trb0a if half == 0 else trb0b
            drain_relu(r0ss[sb][:, 512 * half:512 * (half + 1)],
                       r0p[:, 512 * half:512 * (half + 1)],
                       bias[:, 0:1], _VARIANT["relu0_lanes"][half])

        def s_relu0q(sb, half, ch):  # one [128,256] quarter of relu0
            r0p = state[("r0p", sb)]
            if sb not in r0ss:
                r0ss[sb] = pr0.tile([128, 1024], f32r, tag="r0",
                                    name=f"r0s{sb}")
            bias = trb0a if half == 0 else trb0b
            o = 512 * half + 256 * ch
            drain_relu(r0ss[sb][:, o:o + 256], r0p[:, o:o + 256],
                       bias[:, 0:1], _VARIANT["relu0_lanes"][half])

        def s_rho1T(sb, c0=0, cn=SB):
            # transposed rho1: out[batch, feat] = r0s_blk^T @ rw1 + 1 x rb1.
            # outT reuses the drained acc(sb) PSUM region (dead after
            # s_summ), so it costs no PSUM slot and the acc pool alternates
            # cleanly between consecutive super-blocks.
            r0s = r0ss[sb]
            outT = accs[sb][0:64, 0:128]
            for blk in range(c0 // 64, (c0 + cn) // 64):
                reg = outT[:, 16 * blk:16 * (blk + 1)]
                b0 = 64 * blk
                observe("pe", nc.tensor.matmul(
                    reg, tones, trb1row, start=True, stop=False))
                observe("pe", nc.tensor.matmul(
                    reg, r0s[:, b0:b0 + 64], trw1a,
                    start=False, stop=False))
                observe("pe", nc.tensor.matmul(
                    reg, r0s[:, 512 + b0:512 + b0 + 64], trw1b,
                    start=False, stop=True))

        def s_out(sb, c0=0, cn=SB):
            outT = accs[sb][0:64, 0:128]
            b0, b1 = c0 // 64, (c0 + cn) // 64
            observe("act", nc.scalar.copy(
                oN[:, 128 * sb + 16 * b0:128 * sb + 16 * b1],
                outT[:, 16 * b0:16 * b1]))
            nc.sync.dma_start(
                yv[:, 8 * sb + b0:8 * sb + b1, :],
                oN[:, 128 * sb + 16 * b0:128 * sb + 16 * b1].rearrange(
                    "p (b f) -> p b f", f=16))

        # Newest work first each step (A/B, then C/D, then E) keeps the
        # in-order PE queue from blocking on lane results.  At super-block
        # boundaries E + s_summ are hoisted to the front instead so s_summ
        # lands in its lane queue ahead of the step's main drains and the
        # acc PSUM slot frees before the next super-block's first L2.
        def emit_rho_queue(sb):
            deferred.extend([None] * _VARIANT["rho_spacer"])
            if sb < n_sb - 1 and _VARIANT["rho_fine"]:
                # interior super-blocks: fine-grained pops so the rho PE/ACT
                # work never bursts into the steady A/C stream
                h = SB // 2
                if _VARIANT["relu0_quarters"]:
                    deferred.extend([
                        lambda: s_rho0(sb),
                        lambda: s_relu0q(sb, 0, 0),
                        lambda: s_relu0q(sb, 0, 1),
                        lambda: s_relu0q(sb, 1, 0),
                        lambda: s_relu0q(sb, 1, 1),
                        lambda: s_rho1T(sb, 0, h),
                        lambda: s_rho1T(sb, h, h),
                        lambda: s_out(sb),
                    ])
                else:
                    deferred.extend([
                        lambda: s_rho0(sb),
                        lambda: s_relu0h(sb, 0),
                        lambda: s_relu0h(sb, 1),
                        lambda: s_rho1T(sb, 0, h),
                        lambda: s_rho1T(sb, h, h),
                        lambda: s_out(sb),
                    ])
            else:
                deferred.extend([
                    lambda: s_rho0(sb),
                    lambda: s_relu0(sb),
                    lambda: s_rho1T(sb),
                    lambda: s_out(sb),
                ])

        B_STEAL = _VARIANT["b_steal"]
        stolen = []
        for step in range(n_groups + 2):
            if step < n_groups:
                emit_A(step)
                if step not in B_STEAL:
                    emit_B(step)
            if stolen:  # one-step-delayed h1 drain, rebalanced cross-lane
                emit_D(stolen.pop(), eng=_VARIANT["steal_lane"])
            if 1 <= step <= n_groups:
                emit_C(step - 1)
                if step - 1 in STEAL:
                    stolen.append(step - 1)
                else:
                    emit_D(step - 1)
            if step < n_groups and step in B_STEAL:
                # stolen h0 drain: cross-lane, after the D of this step so
                # the receiving lane never parks on the fresh A matmuls
                emit_B(step, eng=_VARIANT["steal_lane"])
            if step >= 2:
                g = step - 2
                emit_E(g)
                if g % 16 == 15:
                    s_summ(g // 16)
                    emit_rho_queue(g // 16)
            if deferred and step % _VARIANT["pop_every"] == 0:
                fn = deferred.pop(0)
                if fn is not None:
                    fn()
        while deferred:
            fn = deferred.pop(0)
            if fn is not None:
                fn()

    nc.compile()
    return nc


def prep_inputs(inputs):
    """Host-side layout prep: transposed/quantized x + packed weights."""
    f = np.float32

    def q8(a):
        return np.asarray(a, f).astype(F8NP)

    w0 = np.asarray(inputs["phi_w0"], f) * W0SCALE   # [16, 128]
    wAf = np.zeros((128, WA_COLS), f)
    for m in range(8):
        var = np.zeros((128, 128), f)
        var[16 * m:16 * m + 16, :] = w0
        hi = q8(var)
        res = q8(var - hi.astype(f))
        wAf[:, 256 * m:256 * m + 128] = hi.astype(f)
        wAf[:, 256 * m + 128:256 * m + 256] = res.astype(f)
    w2 = np.asarray(inputs["phi_w2"], f) * W2SCALE   # [128, 64]
    hi2 = q8(w2)
    res2 = q8(w2 - hi2.astype(f))
    wAf[:, 2048:2112] = hi2.astype(f)
    wAf[:, 2112:2176] = res2.astype(f)
    w1 = np.asarray(inputs["phi_w1"], f) * W1SCALE   # [128, 128]
    hi1 = q8(w1)
    res1 = q8(w1 - hi1.astype(f))
    wAf[:, 2176:2304] = hi1.astype(f)
    wAf[:, 2304:2432] = res1.astype(f)
    wA = wAf.astype(F8NP)

    rho_w0 = np.asarray(inputs["rho_w0"], f)
    rho_w0 = np.concatenate([rho_w0[32:96], rho_w0[0:32]], axis=0)
    rho_w1 = np.asarray(inputs["rho_w1"], f)
    partsD = {
        "rw0a": rho_w0[:, :128],
        "rw0b": rho_w0[:, 128:],
        "rw1a": rho_w1[:128],
        "rw1b": rho_w1[128:],
        "ones64": np.ones((1, 64), f),
        "rb1row": np.asarray(inputs["rho_b1"], f).reshape(1, 16),
    }
    partsC = {
        "b0s": (W0SCALE * np.asarray(inputs["phi_b0"], f)).reshape(128, 1),
        "b1s": (W0SCALE * W1SCALE
                * np.asarray(inputs["phi_b1"], f)).reshape(128, 1),
        "b2s": (N_NEIGH * np.asarray(inputs["phi_b2"], f)).reshape(HIDDEN, 1),
        "rb0a": np.asarray(inputs["rho_b0"], f)[:128].reshape(128, 1),
        "rb0b": np.asarray(inputs["rho_b0"], f)[128:].reshape(128, 1),
    }
    wCm = np.zeros((128, WC_COLS), f)
    for name, (c0, c1, p) in _WOFF_C.items():
        arr = partsC[name]
        assert arr.shape == (p, c1 - c0), (name, arr.shape)
        wCm[:p, c0:c1] = arr
    wDm = np.zeros((128, WD_COLS), f)
    for name, (c0, c1, p) in _WOFF_D.items():
        arr = partsD[name]
        assert arr.shape == (p, c1 - c0), (name, arr.shape)
        wDm[:p, c0:c1] = arr
    wts = {"wA": wA, "wC": wCm, "wD": wDm}

    x = np.asarray(inputs["x"], f)
    assert x.shape == (B_FULL, XCOLS)
    in_maps = []
    for c in range(N_CORES):
        xT = np.ascontiguousarray(x[c * BC:(c + 1) * BC].T)  # [544, BC]
        in_maps.append({
            "xs8": np.ascontiguousarray(xT[32:544]).astype(F8NP),
            "xg": np.ascontiguousarray(xT[0:32]),
            **wts,
        })
    return in_maps


def prep_weights(inputs):  # kept for test.py compatibility
    return prep_inputs(inputs)[0]


def kernel(**inputs):
    from concourse.bass_utils import run_bass_kernel_spmd

    if "nc" not in _CACHE:
        _CACHE["nc"] = build_nc(NSB_FULL)
    nc = _CACHE["nc"]

    in_maps = prep_inputs(inputs)
    res = run_bass_kernel_spmd(nc, in_maps, list(range(N_CORES)))
    out = np.concatenate([res.results[c]["y"] for c in range(N_CORES)], axis=0)
    return out.astype(np.float32)
